# revision 24
# baseline (speedup 1.0000x reference)
"""Multi-head self-attention (B=2, S=2048, D=1024, H=16, causal) on 8 TRN2 NeuronCores.

Sharding: data parallel over batch (2) x tensor parallel over heads (4 groups of 4).
Core c handles batch c//4, heads 4*(c%4) .. 4*(c%4)+4.
Each core computes a partial output [2048, 1024] (its heads' contribution through
the output projection); the host sums the 4 partials per batch.

Numerics: Q/K projections use 3-term bf16 hi/lo matmuls (exact-ish q,k).
Scores use 2 packed bf16 passes per chunk: per-head partition-stacked tiles
[q_hi; q_lo] x [k_hi; k_hi] and [q_hi; q_lo] x [k_lo; 0] computed via
SBUF->SBUF rearrangement DMAs (DMA can cross partitions; engines cannot).
Softmax in fp32 (DVE row-max + diag mask add, scalar-engine exp with fused
row-sum); attn@V and output projection in bf16 with fp32 accumulation;
fp32 partial outputs.
"""
import sys
for _p in ("/opt/trn_rl_repo", "/root/.axon_site/_ro/trn_rl_repo"):
    if _p not in sys.path:
        sys.path.append(_p)

import math
from contextlib import ExitStack

import numpy as np
import ml_dtypes

import concourse.bass as bass
import concourse.bacc as bacc
import concourse.tile as tile
import concourse.mybir as mybir
from concourse.bass_utils import run_bass_kernel_spmd

BF16 = mybir.dt.bfloat16
F32 = mybir.dt.float32
SEQ = 2048
DM = 1024
DL = 256          # local head dims (4 heads x 64)
DH = 64
MC = 8            # 128-row chunks of the model dim
NQT = SEQ // 128  # 16 q tiles
NEG = -3.0e30
SCW = 512         # score chunk width (PSUM tile, 1 bank)

_CACHE = {}


def build_nc(s_bufs=4, ptp_bufs=2, misc_bufs=2, work_bufs=2, small_bufs=8,
             qkst_bufs=1, xblk_bufs=3, PTG=8):
    nc = bacc.Bacc("TRN2", debug=False, num_devices=8)

    xh_d = nc.dram_tensor("xh", [128, MC, SEQ], BF16, kind="ExternalInput")
    xl_d = nc.dram_tensor("xl", [128, MC, SEQ], BF16, kind="ExternalInput")
    w_d = {}
    for nm_ in ("wqh", "wql", "wkh", "wkl", "wvh"):
        w_d[nm_] = nc.dram_tensor(nm_, [128, MC, DL], BF16, kind="ExternalInput")
    poT_d = nc.dram_tensor("poT", [128, 2, DM], BF16, kind="ExternalInput")
    mask_d = nc.dram_tensor("mask", [128, 128], BF16, kind="ExternalInput")
    ident_d = nc.dram_tensor("ident", [128, 128], BF16, kind="ExternalInput")
    out_d = nc.dram_tensor("out_part", [SEQ, DM], F32, kind="ExternalOutput")

    with tile.TileContext(nc) as tc, ExitStack() as ctx:
        cst = ctx.enter_context(tc.tile_pool(name="cst", bufs=1))
        qkst = ctx.enter_context(tc.tile_pool(name="qkst", bufs=qkst_bufs))
        xblk = ctx.enter_context(tc.tile_pool(name="xblk", bufs=xblk_bufs))
        work = ctx.enter_context(tc.tile_pool(name="work", bufs=work_bufs))
        pwork = ctx.enter_context(tc.tile_pool(name="pwork", bufs=10))
        ptwork = ctx.enter_context(tc.tile_pool(name="ptwork", bufs=4))
        small = ctx.enter_context(tc.tile_pool(name="small", bufs=small_bufs))
        sp = ctx.enter_context(tc.tile_pool(name="sp", bufs=s_bufs, space="PSUM"))
        ptp = ctx.enter_context(tc.tile_pool(name="ptp", bufs=ptp_bufs, space="PSUM"))
        misc = ctx.enter_context(tc.tile_pool(name="misc", bufs=misc_bufs, space="PSUM"))

        # ---- persistent SBUF loads (weights on the gpsimd queue: cheap issue) ----
        wsb = {}
        for nm_ in ("wqh", "wql", "wkh", "wkl", "wvh"):
            t = cst.tile([128, MC, DL], BF16, tag=nm_, name=nm_)
            nc.sync.dma_start(out=t[:, :, :], in_=w_d[nm_][:, :, :])
            wsb[nm_] = t
        poT_sb = cst.tile([128, 2, DM], BF16, tag="poT")
        nc.sync.dma_start(out=poT_sb[:, :, :], in_=poT_d[:, :, :])
        mask_sb = cst.tile([128, 128], BF16, tag="mask")
        nc.sync.dma_start(out=mask_sb, in_=mask_d[:, :])
        ident_sb = cst.tile([128, 128], BF16, tag="ident")
        nc.sync.dma_start(out=ident_sb, in_=ident_d[:, :])

        # ---- per-head stacked score operand tiles ----
        # qhl[dc][e] = [q_hi(64); q_lo(64)] for head 2*dc+e
        # khh[dc][e] = [k_hi(64); k_hi(64)]   (duplicated)
        # klz[dc][e] = [k_lo(64); zeros(64)]
        qhl = [[cst.tile([128, SEQ], BF16, tag=f"qhl{dc}{e}", name=f"qhl{dc}{e}")
                for e in range(2)] for dc in range(2)]
        khh = [[cst.tile([128, SEQ], BF16, tag=f"khh{dc}{e}", name=f"khh{dc}{e}")
                for e in range(2)] for dc in range(2)]
        klz = [[cst.tile([128, SEQ], BF16, tag=f"klz{dc}{e}", name=f"klz{dc}{e}")
                for e in range(2)] for dc in range(2)]
        for dc in range(2):
            for e in range(2):
                nc.gpsimd.memset(klz[dc][e][64:128, :], 0.0)
        v_sb = cst.tile([128, NQT, DL], BF16, tag="v")
        ones_sb = cst.tile([128, 1], BF16, tag="ones")
        nc.gpsimd.memset(ones_sb[:, :], 1.0)

        def load_x_block(qc):
            """Stream one 512-col block of x (all MC chunks) on the SP queue."""
            xbh = xblk.tile([128, MC, 512], BF16, tag="xbh", name=f"xbh{qc}")
            xbl = xblk.tile([128, MC, 512], BF16, tag="xbl", name=f"xbl{qc}")
            nc.sync.dma_start(out=xbh[:, :, :], in_=xh_d[:, :, 512 * qc:512 * (qc + 1)])
            nc.sync.dma_start(out=xbl[:, :, :], in_=xl_d[:, :, 512 * qc:512 * (qc + 1)])
            return xbh, xbl

        def emit_proj_chunk(qc, xbh, xbl):
            cols = slice(512 * qc, 512 * (qc + 1))
            for wh_nm, wl_nm, dst_q in (("wqh", "wql", True), ("wkh", "wkl", False)):
                st_h = qkst.tile([128, 2, 512], BF16, tag=f"sh{wh_nm}", name="st_h")
                st_l = qkst.tile([128, 2, 512], BF16, tag=f"sl{wh_nm}", name="st_l")
                for dc in range(2):
                    ps = sp.tile([128, SCW], F32, tag="s", name="ps")
                    n = 0
                    for m in range(MC):
                        for lt, rt in ((wsb[wh_nm], xbh), (wsb[wh_nm], xbl),
                                       (wsb[wl_nm], xbh)):
                            nc.tensor.matmul(
                                ps[:, :512],
                                lt[:, m, 128 * dc:128 * (dc + 1)],
                                rt[:, m, :],
                                start=(n == 0), stop=(n == 3 * MC - 1))
                            n += 1
                    nc.scalar.copy(out=st_h[:, dc, :], in_=ps[:, :512])
                    nc.vector.tensor_sub(st_l[:, dc, :], ps[:, :512], st_h[:, dc, :])
                # rearrangement DMAs into stacked per-head tiles (gpsimd queue)
                for dc in range(2):
                    for e in range(2):
                        hs = slice(64 * e, 64 * (e + 1))
                        if dst_q:
                            nc.gpsimd.dma_start(out=qhl[dc][e][0:64, cols],
                                                in_=st_h[hs, dc, :])
                            nc.gpsimd.dma_start(out=qhl[dc][e][64:128, cols],
                                                in_=st_l[hs, dc, :])
                        else:
                            nc.gpsimd.dma_start(out=khh[dc][e][0:64, cols],
                                                in_=st_h[hs, dc, :])
                            nc.gpsimd.dma_start(out=khh[dc][e][64:128, cols],
                                                in_=st_h[hs, dc, :])
                            nc.gpsimd.dma_start(out=klz[dc][e][0:64, cols],
                                                in_=st_l[hs, dc, :])

        def emit_v(st, xbh):
            ps = misc.tile([128, 512], F32, tag="m", name="vps")
            for m in range(MC):
                nc.tensor.matmul(ps[:, :DL], xbh[:, m, 128 * (st % 4):128 * (st % 4) + 128],
                                 wsb["wvh"][:, m, :], start=(m == 0), stop=(m == MC - 1))
            nc.scalar.copy(out=v_sb[:, st, :], in_=ps[:, :DL])

        # ---- attention (software-pipelined: back-half of q-tile qi is
        # emitted after the front-half of qi+1 so the PE never stalls on a
        # fresh softmax) ----
        xblocks = [load_x_block(qc) for qc in range(4)]

        def emit_front(qi):
            if qi % 4 == 0:
                xbh, xbl = xblocks[qi // 4]
                emit_proj_chunk(qi // 4, xbh, xbl)
                emit_front.xbh = xbh
            emit_v(qi, emit_front.xbh)
            nkt = qi + 1          # causal k tiles
            kend = nkt * 128
            attn_cat = work.tile([128, DL], BF16, tag="acat")
            streams = []
            for hp in range(2):
                for e in range(2):
                    p_sb = pwork.tile([128, SEQ], BF16, tag="p", name="p_sb")
                    ncw = (kend + SCW - 1) // SCW
                    s_tiles = [None] * ncw
                    nms = []
                    qcols = slice(128 * qi, 128 * (qi + 1))
                    for ck in range(ncw):
                        cw = min(SCW, kend - SCW * ck)
                        sps = sp.tile([128, SCW], F32, tag="s", name="s_ps")
                        s_tiles[ck] = sps
                        has_mask = (SCW * ck <= 128 * qi < SCW * ck + cw)
                        for sub in range((cw + 511) // 512):
                            sw = min(512, cw - 512 * sub)
                            kcols = slice(SCW * ck + 512 * sub, SCW * ck + 512 * sub + sw)
                            nc.tensor.matmul(sps[:, 512 * sub:512 * sub + sw],
                                             qhl[hp][e][:, qcols], khh[hp][e][:, kcols],
                                             start=True, stop=False)
                            nc.tensor.matmul(sps[:, 512 * sub:512 * sub + sw],
                                             qhl[hp][e][:, qcols], klz[hp][e][:, kcols],
                                             start=False, stop=not has_mask)
                        if has_mask:
                            off = 128 * qi - SCW * ck
                            nc.tensor.matmul(sps[:, off:off + 128],
                                             ident_sb[:, :], mask_sb[:, :],
                                             start=False, stop=True)
                        nmc = small.tile([128, 1], F32, tag="nmc", name="nmc")
                        nc.vector.tensor_reduce(out=nmc, in_=sps[:, :cw],
                                                axis=mybir.AxisListType.X,
                                                op=mybir.AluOpType.max, negate=True)
                        nms.append(nmc)
                    nm = nms[0]
                    for ck in range(1, ncw):
                        nmg = small.tile([128, 1], F32, tag="nmg", name="nmg")
                        nc.vector.tensor_tensor(out=nmg, in0=nm, in1=nms[ck],
                                                op=mybir.AluOpType.min)
                        nm = nmg
                    for ck in range(ncw):
                        cw = min(SCW, kend - SCW * ck)
                        nc.scalar.activation(out=p_sb[:, SCW * ck:SCW * ck + cw],
                                             in_=s_tiles[ck][:, :cw],
                                             func=mybir.ActivationFunctionType.Exp,
                                             bias=nm, scale=1.0)
                    streams.append((hp, e, p_sb))
            return qi, nkt, attn_cat, streams

        def emit_back(ctx):
            qi, nkt, attn_cat, streams = ctx
            av_pairs = {}
            for hp, e, p_sb in streams:
                    h_local = 2 * hp + e
                    # P^T via PE transpose, PTG k-tiles per group
                    pt_sb = ptwork.tile([128, SEQ], BF16, tag="pt", name="pt_sb")
                    for g in range((nkt + PTG - 1) // PTG):
                        n_in_g = min(PTG, nkt - PTG * g)
                        ptps = ptp.tile([128, 128 * PTG], BF16, tag="ptps", name="ptps")
                        for j in range(n_in_g):
                            kt_i = PTG * g + j
                            nc.tensor.transpose(ptps[:, 128 * j:128 * (j + 1)],
                                                p_sb[:, 128 * kt_i:128 * (kt_i + 1)],
                                                ident_sb)
                        if g % 2 == 0:
                            nc.vector.tensor_copy(out=pt_sb[:, 128 * PTG * g:128 * PTG * g + 128 * n_in_g],
                                                  in_=ptps[:, :128 * n_in_g])
                        else:
                            nc.scalar.copy(out=pt_sb[:, 128 * PTG * g:128 * PTG * g + 128 * n_in_g],
                                           in_=ptps[:, :128 * n_in_g])
                    # attn @ V with fp32 accumulation
                    if e == 0:
                        av_pairs[hp] = misc.tile([128, 512], F32, tag="m", name="av_pair")
                    av_pair = av_pairs[hp]
                    for kt_i in range(nkt):
                        nc.tensor.matmul(av_pair[:, 65 * e:65 * e + 64],
                                         pt_sb[:, 128 * kt_i:128 * (kt_i + 1)],
                                         v_sb[:, kt_i, 64 * h_local:64 * (h_local + 1)],
                                         start=(kt_i == 0), stop=(kt_i == nkt - 1))
                    for kt_i in range(nkt):
                        nc.tensor.matmul(av_pair[:, 65 * e + 64:65 * e + 65],
                                         pt_sb[:, 128 * kt_i:128 * (kt_i + 1)],
                                         ones_sb[:, :],
                                         start=(kt_i == 0), stop=(kt_i == nkt - 1))
                    inv = small.tile([128, 1], F32, tag="inv", name="inv")
                    nc.vector.reciprocal(out=inv, in_=av_pair[:, 65 * e + 64:65 * e + 65])
                    nc.scalar.activation(
                        out=attn_cat[:, 64 * h_local:64 * (h_local + 1)],
                        in_=av_pair[:, 65 * e:65 * e + 64],
                        func=mybir.ActivationFunctionType.Copy, scale=inv)
            # ---- output projection for this q tile ----
            acT_ps = ptp.tile([128, 128 * PTG], BF16, tag="ptps", name="acT_ps")
            nc.tensor.transpose(acT_ps[:, 0:128], attn_cat[:, 0:128], ident_sb)
            nc.tensor.transpose(acT_ps[:, 128:256], attn_cat[:, 128:256], ident_sb)
            acT = work.tile([128, 256], BF16, tag="acT")
            nc.vector.tensor_copy(out=acT[:, :], in_=acT_ps[:, :256])
            out_sb = work.tile([128, DM], F32, tag="osb")
            for nc_i in range(2):
                ops = misc.tile([128, 512], F32, tag="m", name="ops")
                for mlc in range(2):
                    nc.tensor.matmul(ops[:, :512], acT[:, 128 * mlc:128 * (mlc + 1)],
                                     poT_sb[:, mlc, 512 * nc_i:512 * (nc_i + 1)],
                                     start=(mlc == 0), stop=(mlc == 1))
                if nc_i == 0:
                    nc.scalar.copy(out=out_sb[:, 0:512], in_=ops[:, :512])
                else:
                    nc.vector.tensor_copy(out=out_sb[:, 512:1024], in_=ops[:, :512])
            nc.gpsimd.dma_start(out=out_d[128 * qi:128 * (qi + 1), :], in_=out_sb)

        prev = None
        for qi in range(NQT):
            ctx_f = emit_front(qi)
            if prev is not None:
                emit_back(prev)
            prev = ctx_f
        emit_back(prev)

    nc.compile()
    return nc


def _bf16(a):
    return a.astype(ml_dtypes.bfloat16)


def _split(a):
    hi = _bf16(a)
    lo = _bf16(a - hi.astype(np.float32))
    return hi, lo


def _prep_inputs(x, p_q, p_k, p_v, p_o):
    """Build the 8 per-core input maps."""
    per_batch = []
    for b in range(2):
        xT = np.ascontiguousarray(x[b].T).astype(np.float32)  # [1024, 2048]
        xh, xl = _split(xT)
        per_batch.append((np.ascontiguousarray(xh.reshape(MC, 128, SEQ).transpose(1, 0, 2)),
                          np.ascontiguousarray(xl.reshape(MC, 128, SEQ).transpose(1, 0, 2))))

    mask = np.zeros((128, 128), np.float32)
    iu = np.triu_indices(128, 1)
    mask[iu] = NEG
    mask = _bf16(mask)
    ident = np.eye(128, dtype=ml_dtypes.bfloat16)

    per_group = []
    for g in range(4):
        rows = slice(DL * g, DL * (g + 1))
        wqT = np.ascontiguousarray((p_q[rows] / math.sqrt(DH)).T).astype(np.float32)
        wkT = np.ascontiguousarray(p_k[rows].T).astype(np.float32)
        wvT = np.ascontiguousarray(p_v[rows].T).astype(np.float32)
        poT = np.ascontiguousarray(p_o[:, rows].T).astype(np.float32)
        wqh, wql = _split(wqT)
        wkh, wkl = _split(wkT)
        def _pm(a):
            return np.ascontiguousarray(a.reshape(MC, 128, DL).transpose(1, 0, 2))
        per_group.append(dict(
            wqh=_pm(wqh), wql=_pm(wql), wkh=_pm(wkh), wkl=_pm(wkl),
            wvh=_pm(_bf16(wvT)),
            poT=np.ascontiguousarray(_bf16(poT).reshape(2, 128, DM).transpose(1, 0, 2)),
        ))

    in_maps = []
    for c in range(8):
        b, g = c // 4, c % 4
        m = dict(per_group[g])
        m["xh"], m["xl"] = per_batch[b]
        m["mask"] = mask
        m["ident"] = ident
        in_maps.append(m)
    return in_maps


def kernel(x, p_q, p_k, p_v, p_o):
    if "nc" not in _CACHE:
        _CACHE["nc"] = build_nc()
    nc = _CACHE["nc"]
    in_maps = _prep_inputs(np.asarray(x), np.asarray(p_q), np.asarray(p_k),
                           np.asarray(p_v), np.asarray(p_o))
    res = run_bass_kernel_spmd(nc, in_maps, core_ids=list(range(8)))
    parts = [r["out_part"].astype(np.float32) for r in res.results]
    out = np.stack([parts[0] + parts[1] + parts[2] + parts[3],
                    parts[4] + parts[5] + parts[6] + parts[7]])
    return out.astype(np.float32)


# revision 25
# speedup vs baseline: 1.0047x; 1.0047x over previous
"""Multi-head self-attention (B=2, S=2048, D=1024, H=16, causal) on 8 TRN2 NeuronCores.

Sharding: data parallel over batch (2) x tensor parallel over heads (4 groups of 4).
Core c handles batch c//4, heads 4*(c%4) .. 4*(c%4)+4.
Each core computes a partial output [2048, 1024] (its heads' contribution through
the output projection); the host sums the 4 partials per batch.

Numerics: Q/K projections use 3-term bf16 hi/lo matmuls (exact-ish q,k).
Scores use 2 packed bf16 passes per chunk: per-head partition-stacked tiles
[q_hi; q_lo] x [k_hi; k_hi] and [q_hi; q_lo] x [k_lo; 0] computed via
SBUF->SBUF rearrangement DMAs (DMA can cross partitions; engines cannot).
Softmax in fp32 (DVE row-max + diag mask add, scalar-engine exp with fused
row-sum); attn@V and output projection in bf16 with fp32 accumulation;
fp32 partial outputs.
"""
import sys
for _p in ("/opt/trn_rl_repo", "/root/.axon_site/_ro/trn_rl_repo"):
    if _p not in sys.path:
        sys.path.append(_p)

import math
from contextlib import ExitStack

import numpy as np
import ml_dtypes

import concourse.bass as bass
import concourse.bacc as bacc
import concourse.tile as tile
import concourse.mybir as mybir
from concourse.bass_utils import run_bass_kernel_spmd

BF16 = mybir.dt.bfloat16
F32 = mybir.dt.float32
SEQ = 2048
DM = 1024
DL = 256          # local head dims (4 heads x 64)
DH = 64
MC = 8            # 128-row chunks of the model dim
NQT = SEQ // 128  # 16 q tiles
NEG = -3.0e30
SCW = 512         # score chunk width (PSUM tile, 1 bank)

_CACHE = {}


def build_nc(s_bufs=4, ptp_bufs=2, misc_bufs=2, work_bufs=2, small_bufs=8,
             qkst_bufs=1, xblk_bufs=4, PTG=8):
    nc = bacc.Bacc("TRN2", debug=False, num_devices=8)

    xh_d = nc.dram_tensor("xh", [128, MC, SEQ], BF16, kind="ExternalInput")
    xl_d = nc.dram_tensor("xl", [128, MC, SEQ], BF16, kind="ExternalInput")
    w_d = {}
    for nm_ in ("wqh", "wql", "wkh", "wkl", "wvh"):
        w_d[nm_] = nc.dram_tensor(nm_, [128, MC, DL], BF16, kind="ExternalInput")
    poT_d = nc.dram_tensor("poT", [128, 2, DM], BF16, kind="ExternalInput")
    mask_d = nc.dram_tensor("mask", [128, 128], BF16, kind="ExternalInput")
    ident_d = nc.dram_tensor("ident", [128, 128], BF16, kind="ExternalInput")
    out_d = nc.dram_tensor("out_part", [SEQ, DM], F32, kind="ExternalOutput")

    with tile.TileContext(nc) as tc, ExitStack() as ctx:
        cst = ctx.enter_context(tc.tile_pool(name="cst", bufs=1))
        qkst = ctx.enter_context(tc.tile_pool(name="qkst", bufs=qkst_bufs))
        xblk = ctx.enter_context(tc.tile_pool(name="xblk", bufs=xblk_bufs))
        work = ctx.enter_context(tc.tile_pool(name="work", bufs=work_bufs))
        pwork = ctx.enter_context(tc.tile_pool(name="pwork", bufs=8))
        ptwork = ctx.enter_context(tc.tile_pool(name="ptwork", bufs=3))
        small = ctx.enter_context(tc.tile_pool(name="small", bufs=small_bufs))
        sp = ctx.enter_context(tc.tile_pool(name="sp", bufs=s_bufs, space="PSUM"))
        ptp = ctx.enter_context(tc.tile_pool(name="ptp", bufs=ptp_bufs, space="PSUM"))
        misc = ctx.enter_context(tc.tile_pool(name="misc", bufs=misc_bufs, space="PSUM"))

        # ---- persistent SBUF loads (weights on the gpsimd queue: cheap issue) ----
        wsb = {}
        for nm_ in ("wqh", "wql", "wkh", "wkl", "wvh"):
            t = cst.tile([128, MC, DL], BF16, tag=nm_, name=nm_)
            nc.sync.dma_start(out=t[:, :, :], in_=w_d[nm_][:, :, :])
            wsb[nm_] = t
        poT_sb = cst.tile([128, 2, DM], BF16, tag="poT")
        nc.sync.dma_start(out=poT_sb[:, :, :], in_=poT_d[:, :, :])
        mask_sb = cst.tile([128, 128], BF16, tag="mask")
        nc.sync.dma_start(out=mask_sb, in_=mask_d[:, :])
        ident_sb = cst.tile([128, 128], BF16, tag="ident")
        nc.sync.dma_start(out=ident_sb, in_=ident_d[:, :])

        # ---- per-head stacked score operand tiles ----
        # qhl[dc][e] = [q_hi(64); q_lo(64)] for head 2*dc+e
        # khh[dc][e] = [k_hi(64); k_hi(64)]   (duplicated)
        # klz[dc][e] = [k_lo(64); zeros(64)]
        qhl = [[cst.tile([128, SEQ], BF16, tag=f"qhl{dc}{e}", name=f"qhl{dc}{e}")
                for e in range(2)] for dc in range(2)]
        khh = [[cst.tile([128, SEQ], BF16, tag=f"khh{dc}{e}", name=f"khh{dc}{e}")
                for e in range(2)] for dc in range(2)]
        klz = [[cst.tile([128, SEQ], BF16, tag=f"klz{dc}{e}", name=f"klz{dc}{e}")
                for e in range(2)] for dc in range(2)]
        for dc in range(2):
            for e in range(2):
                nc.gpsimd.memset(klz[dc][e][64:128, :], 0.0)
        v_sb = cst.tile([128, NQT, DL], BF16, tag="v")
        ones_sb = cst.tile([128, 1], BF16, tag="ones")
        nc.gpsimd.memset(ones_sb[:, :], 1.0)

        def load_x_block(qc):
            """Stream one 512-col block of x (all MC chunks) on the SP queue."""
            xbh = xblk.tile([128, MC, 512], BF16, tag="xbh", name=f"xbh{qc}")
            xbl = xblk.tile([128, MC, 512], BF16, tag="xbl", name=f"xbl{qc}")
            nc.sync.dma_start(out=xbh[:, :, :], in_=xh_d[:, :, 512 * qc:512 * (qc + 1)])
            nc.sync.dma_start(out=xbl[:, :, :], in_=xl_d[:, :, 512 * qc:512 * (qc + 1)])
            return xbh, xbl

        def emit_proj_chunk(qc, xbh, xbl):
            cols = slice(512 * qc, 512 * (qc + 1))
            for wh_nm, wl_nm, dst_q in (("wqh", "wql", True), ("wkh", "wkl", False)):
                st_h = qkst.tile([128, 2, 512], BF16, tag=f"sh{wh_nm}", name="st_h")
                st_l = qkst.tile([128, 2, 512], BF16, tag=f"sl{wh_nm}", name="st_l")
                for dc in range(2):
                    ps = sp.tile([128, SCW], F32, tag="s", name="ps")
                    n = 0
                    for m in range(MC):
                        for lt, rt in ((wsb[wh_nm], xbh), (wsb[wh_nm], xbl),
                                       (wsb[wl_nm], xbh)):
                            nc.tensor.matmul(
                                ps[:, :512],
                                lt[:, m, 128 * dc:128 * (dc + 1)],
                                rt[:, m, :],
                                start=(n == 0), stop=(n == 3 * MC - 1))
                            n += 1
                    nc.scalar.copy(out=st_h[:, dc, :], in_=ps[:, :512])
                    nc.vector.tensor_sub(st_l[:, dc, :], ps[:, :512], st_h[:, dc, :])
                # rearrangement DMAs into stacked per-head tiles (gpsimd queue)
                for dc in range(2):
                    for e in range(2):
                        hs = slice(64 * e, 64 * (e + 1))
                        if dst_q:
                            nc.gpsimd.dma_start(out=qhl[dc][e][0:64, cols],
                                                in_=st_h[hs, dc, :])
                            nc.gpsimd.dma_start(out=qhl[dc][e][64:128, cols],
                                                in_=st_l[hs, dc, :])
                        else:
                            nc.gpsimd.dma_start(out=khh[dc][e][0:64, cols],
                                                in_=st_h[hs, dc, :])
                            nc.gpsimd.dma_start(out=khh[dc][e][64:128, cols],
                                                in_=st_h[hs, dc, :])
                            nc.gpsimd.dma_start(out=klz[dc][e][0:64, cols],
                                                in_=st_l[hs, dc, :])

        def emit_v(st, xbh):
            ps = misc.tile([128, 512], F32, tag="m", name="vps")
            for m in range(MC):
                nc.tensor.matmul(ps[:, :DL], xbh[:, m, 128 * (st % 4):128 * (st % 4) + 128],
                                 wsb["wvh"][:, m, :], start=(m == 0), stop=(m == MC - 1))
            nc.scalar.copy(out=v_sb[:, st, :], in_=ps[:, :DL])

        # ---- attention (software-pipelined: back-half of q-tile qi is
        # emitted after the front-half of qi+1 so the PE never stalls on a
        # fresh softmax) ----
        xblocks = [load_x_block(qc) for qc in range(4)]

        def emit_front(qi):
            if qi % 4 == 0:
                xbh, xbl = xblocks[qi // 4]
                emit_proj_chunk(qi // 4, xbh, xbl)
                emit_front.xbh = xbh
            emit_v(qi, emit_front.xbh)
            nkt = qi + 1          # causal k tiles
            kend = nkt * 128
            attn_cat = work.tile([128, DL], BF16, tag="acat")
            streams = []
            for hp in range(2):
                for e in range(2):
                    p_sb = pwork.tile([128, SEQ], BF16, tag="p", name="p_sb")
                    ncw = (kend + SCW - 1) // SCW
                    s_tiles = [None] * ncw
                    nms = []
                    qcols = slice(128 * qi, 128 * (qi + 1))
                    for ck in range(ncw):
                        cw = min(SCW, kend - SCW * ck)
                        sps = sp.tile([128, SCW], F32, tag="s", name="s_ps")
                        s_tiles[ck] = sps
                        has_mask = (SCW * ck <= 128 * qi < SCW * ck + cw)
                        for sub in range((cw + 511) // 512):
                            sw = min(512, cw - 512 * sub)
                            kcols = slice(SCW * ck + 512 * sub, SCW * ck + 512 * sub + sw)
                            nc.tensor.matmul(sps[:, 512 * sub:512 * sub + sw],
                                             qhl[hp][e][:, qcols], khh[hp][e][:, kcols],
                                             start=True, stop=False)
                            nc.tensor.matmul(sps[:, 512 * sub:512 * sub + sw],
                                             qhl[hp][e][:, qcols], klz[hp][e][:, kcols],
                                             start=False, stop=not has_mask)
                        if has_mask:
                            off = 128 * qi - SCW * ck
                            nc.tensor.matmul(sps[:, off:off + 128],
                                             ident_sb[:, :], mask_sb[:, :],
                                             start=False, stop=True)
                        nmc = small.tile([128, 1], F32, tag="nmc", name="nmc")
                        nc.vector.tensor_reduce(out=nmc, in_=sps[:, :cw],
                                                axis=mybir.AxisListType.X,
                                                op=mybir.AluOpType.max, negate=True)
                        nms.append(nmc)
                    nm = nms[0]
                    for ck in range(1, ncw):
                        nmg = small.tile([128, 1], F32, tag="nmg", name="nmg")
                        nc.vector.tensor_tensor(out=nmg, in0=nm, in1=nms[ck],
                                                op=mybir.AluOpType.min)
                        nm = nmg
                    for ck in range(ncw):
                        cw = min(SCW, kend - SCW * ck)
                        nc.scalar.activation(out=p_sb[:, SCW * ck:SCW * ck + cw],
                                             in_=s_tiles[ck][:, :cw],
                                             func=mybir.ActivationFunctionType.Exp,
                                             bias=nm, scale=1.0)
                    streams.append((hp, e, p_sb))
            return qi, nkt, attn_cat, streams

        def emit_back(ctx):
            qi, nkt, attn_cat, streams = ctx
            av_pairs = {}
            for hp, e, p_sb in streams:
                    h_local = 2 * hp + e
                    # P^T via PE transpose, PTG k-tiles per group
                    pt_sb = ptwork.tile([128, SEQ], BF16, tag="pt", name="pt_sb")
                    for g in range((nkt + PTG - 1) // PTG):
                        n_in_g = min(PTG, nkt - PTG * g)
                        ptps = ptp.tile([128, 128 * PTG], BF16, tag="ptps", name="ptps")
                        for j in range(n_in_g):
                            kt_i = PTG * g + j
                            nc.tensor.transpose(ptps[:, 128 * j:128 * (j + 1)],
                                                p_sb[:, 128 * kt_i:128 * (kt_i + 1)],
                                                ident_sb)
                        if g % 2 == 0:
                            nc.vector.tensor_copy(out=pt_sb[:, 128 * PTG * g:128 * PTG * g + 128 * n_in_g],
                                                  in_=ptps[:, :128 * n_in_g])
                        else:
                            nc.scalar.copy(out=pt_sb[:, 128 * PTG * g:128 * PTG * g + 128 * n_in_g],
                                           in_=ptps[:, :128 * n_in_g])
                    # attn @ V with fp32 accumulation
                    if e == 0:
                        av_pairs[hp] = misc.tile([128, 512], F32, tag="m", name="av_pair")
                    av_pair = av_pairs[hp]
                    for kt_i in range(nkt):
                        nc.tensor.matmul(av_pair[:, 65 * e:65 * e + 64],
                                         pt_sb[:, 128 * kt_i:128 * (kt_i + 1)],
                                         v_sb[:, kt_i, 64 * h_local:64 * (h_local + 1)],
                                         start=(kt_i == 0), stop=(kt_i == nkt - 1))
                    for kt_i in range(nkt):
                        nc.tensor.matmul(av_pair[:, 65 * e + 64:65 * e + 65],
                                         pt_sb[:, 128 * kt_i:128 * (kt_i + 1)],
                                         ones_sb[:, :],
                                         start=(kt_i == 0), stop=(kt_i == nkt - 1))
                    inv = small.tile([128, 1], F32, tag="inv", name="inv")
                    nc.vector.reciprocal(out=inv, in_=av_pair[:, 65 * e + 64:65 * e + 65])
                    nc.scalar.activation(
                        out=attn_cat[:, 64 * h_local:64 * (h_local + 1)],
                        in_=av_pair[:, 65 * e:65 * e + 64],
                        func=mybir.ActivationFunctionType.Copy, scale=inv)
            # ---- output projection for this q tile ----
            acT_ps = ptp.tile([128, 128 * PTG], BF16, tag="ptps", name="acT_ps")
            nc.tensor.transpose(acT_ps[:, 0:128], attn_cat[:, 0:128], ident_sb)
            nc.tensor.transpose(acT_ps[:, 128:256], attn_cat[:, 128:256], ident_sb)
            acT = work.tile([128, 256], BF16, tag="acT")
            nc.vector.tensor_copy(out=acT[:, :], in_=acT_ps[:, :256])
            out_sb = work.tile([128, DM], F32, tag="osb")
            for nc_i in range(2):
                ops = misc.tile([128, 512], F32, tag="m", name="ops")
                for mlc in range(2):
                    nc.tensor.matmul(ops[:, :512], acT[:, 128 * mlc:128 * (mlc + 1)],
                                     poT_sb[:, mlc, 512 * nc_i:512 * (nc_i + 1)],
                                     start=(mlc == 0), stop=(mlc == 1))
                if nc_i == 0:
                    nc.scalar.copy(out=out_sb[:, 0:512], in_=ops[:, :512])
                else:
                    nc.vector.tensor_copy(out=out_sb[:, 512:1024], in_=ops[:, :512])
            nc.gpsimd.dma_start(out=out_d[128 * qi:128 * (qi + 1), :], in_=out_sb)

        prev = None
        for qi in range(NQT):
            ctx_f = emit_front(qi)
            if prev is not None:
                emit_back(prev)
            prev = ctx_f
        emit_back(prev)

    nc.compile()
    return nc


def _bf16(a):
    return a.astype(ml_dtypes.bfloat16)


def _split(a):
    hi = _bf16(a)
    lo = _bf16(a - hi.astype(np.float32))
    return hi, lo


def _prep_inputs(x, p_q, p_k, p_v, p_o):
    """Build the 8 per-core input maps."""
    per_batch = []
    for b in range(2):
        xT = np.ascontiguousarray(x[b].T).astype(np.float32)  # [1024, 2048]
        xh, xl = _split(xT)
        per_batch.append((np.ascontiguousarray(xh.reshape(MC, 128, SEQ).transpose(1, 0, 2)),
                          np.ascontiguousarray(xl.reshape(MC, 128, SEQ).transpose(1, 0, 2))))

    mask = np.zeros((128, 128), np.float32)
    iu = np.triu_indices(128, 1)
    mask[iu] = NEG
    mask = _bf16(mask)
    ident = np.eye(128, dtype=ml_dtypes.bfloat16)

    per_group = []
    for g in range(4):
        rows = slice(DL * g, DL * (g + 1))
        wqT = np.ascontiguousarray((p_q[rows] / math.sqrt(DH)).T).astype(np.float32)
        wkT = np.ascontiguousarray(p_k[rows].T).astype(np.float32)
        wvT = np.ascontiguousarray(p_v[rows].T).astype(np.float32)
        poT = np.ascontiguousarray(p_o[:, rows].T).astype(np.float32)
        wqh, wql = _split(wqT)
        wkh, wkl = _split(wkT)
        def _pm(a):
            return np.ascontiguousarray(a.reshape(MC, 128, DL).transpose(1, 0, 2))
        per_group.append(dict(
            wqh=_pm(wqh), wql=_pm(wql), wkh=_pm(wkh), wkl=_pm(wkl),
            wvh=_pm(_bf16(wvT)),
            poT=np.ascontiguousarray(_bf16(poT).reshape(2, 128, DM).transpose(1, 0, 2)),
        ))

    in_maps = []
    for c in range(8):
        b, g = c // 4, c % 4
        m = dict(per_group[g])
        m["xh"], m["xl"] = per_batch[b]
        m["mask"] = mask
        m["ident"] = ident
        in_maps.append(m)
    return in_maps


def kernel(x, p_q, p_k, p_v, p_o):
    if "nc" not in _CACHE:
        _CACHE["nc"] = build_nc()
    nc = _CACHE["nc"]
    in_maps = _prep_inputs(np.asarray(x), np.asarray(p_q), np.asarray(p_k),
                           np.asarray(p_v), np.asarray(p_o))
    res = run_bass_kernel_spmd(nc, in_maps, core_ids=list(range(8)))
    parts = [r["out_part"].astype(np.float32) for r in res.results]
    out = np.stack([parts[0] + parts[1] + parts[2] + parts[3],
                    parts[4] + parts[5] + parts[6] + parts[7]])
    return out.astype(np.float32)


# revision 27
# speedup vs baseline: 1.0074x; 1.0027x over previous
"""Multi-head self-attention (B=2, S=2048, D=1024, H=16, causal) on 8 TRN2 NeuronCores.

Sharding: data parallel over batch (2) x tensor parallel over heads (4 groups of 4).
Core c handles batch c//4, heads 4*(c%4) .. 4*(c%4)+4.
Each core computes a partial output [2048, 1024] (its heads' contribution through
the output projection); the host sums the 4 partials per batch.

Numerics: Q/K projections use 3-term bf16 hi/lo matmuls (exact-ish q,k).
Scores use 2 packed bf16 passes per chunk: per-head partition-stacked tiles
[q_hi; q_lo] x [k_hi; k_hi] and [q_hi; q_lo] x [k_lo; 0] computed via
SBUF->SBUF rearrangement DMAs (DMA can cross partitions; engines cannot).
Softmax in fp32 (DVE row-max + diag mask add, scalar-engine exp with fused
row-sum); attn@V and output projection in bf16 with fp32 accumulation;
fp32 partial outputs.
"""
import sys
for _p in ("/opt/trn_rl_repo", "/root/.axon_site/_ro/trn_rl_repo"):
    if _p not in sys.path:
        sys.path.append(_p)

import math
from contextlib import ExitStack

import numpy as np
import ml_dtypes

import concourse.bass as bass
import concourse.bacc as bacc
import concourse.tile as tile
import concourse.mybir as mybir
from concourse.bass_utils import run_bass_kernel_spmd

BF16 = mybir.dt.bfloat16
F32 = mybir.dt.float32
SEQ = 2048
DM = 1024
DL = 256          # local head dims (4 heads x 64)
DH = 64
MC = 8            # 128-row chunks of the model dim
NQT = SEQ // 128  # 16 q tiles
NEG = -3.0e30
SCW = 512         # score chunk width (PSUM tile, 1 bank)

_CACHE = {}


def build_nc(s_bufs=4, ptp_bufs=2, misc_bufs=2, work_bufs=2, small_bufs=12,
             qkst_bufs=1, xblk_bufs=3, PTG=8):
    nc = bacc.Bacc("TRN2", debug=False, num_devices=8)

    xh_d = nc.dram_tensor("xh", [128, MC, SEQ], BF16, kind="ExternalInput")
    xl_d = nc.dram_tensor("xl", [128, MC, SEQ], BF16, kind="ExternalInput")
    w_d = {}
    for nm_ in ("wqh", "wql", "wkh", "wkl", "wvh"):
        w_d[nm_] = nc.dram_tensor(nm_, [128, MC, DL], BF16, kind="ExternalInput")
    poT_d = nc.dram_tensor("poT", [128, 2, DM], BF16, kind="ExternalInput")
    mask_d = nc.dram_tensor("mask", [128, 128], BF16, kind="ExternalInput")
    ident_d = nc.dram_tensor("ident", [128, 128], BF16, kind="ExternalInput")
    out_d = nc.dram_tensor("out_part", [SEQ, DM], F32, kind="ExternalOutput")

    with tile.TileContext(nc) as tc, ExitStack() as ctx:
        cst = ctx.enter_context(tc.tile_pool(name="cst", bufs=1))
        qkst = ctx.enter_context(tc.tile_pool(name="qkst", bufs=qkst_bufs))
        xblk = ctx.enter_context(tc.tile_pool(name="xblk", bufs=xblk_bufs))
        work = ctx.enter_context(tc.tile_pool(name="work", bufs=work_bufs))
        pwork = ctx.enter_context(tc.tile_pool(name="pwork", bufs=12))
        ptwork = ctx.enter_context(tc.tile_pool(name="ptwork", bufs=3))
        small = ctx.enter_context(tc.tile_pool(name="small", bufs=small_bufs))
        sp = ctx.enter_context(tc.tile_pool(name="sp", bufs=s_bufs, space="PSUM"))
        ptp = ctx.enter_context(tc.tile_pool(name="ptp", bufs=ptp_bufs, space="PSUM"))
        misc = ctx.enter_context(tc.tile_pool(name="misc", bufs=misc_bufs, space="PSUM"))

        # ---- persistent SBUF loads (weights on the gpsimd queue: cheap issue) ----
        wsb = {}
        for nm_ in ("wqh", "wql", "wkh", "wkl", "wvh"):
            t = cst.tile([128, MC, DL], BF16, tag=nm_, name=nm_)
            nc.sync.dma_start(out=t[:, :, :], in_=w_d[nm_][:, :, :])
            wsb[nm_] = t
        poT_sb = cst.tile([128, 2, DM], BF16, tag="poT")
        nc.sync.dma_start(out=poT_sb[:, :, :], in_=poT_d[:, :, :])
        mask_sb = cst.tile([128, 128], BF16, tag="mask")
        nc.sync.dma_start(out=mask_sb, in_=mask_d[:, :])
        ident_sb = cst.tile([128, 128], BF16, tag="ident")
        nc.sync.dma_start(out=ident_sb, in_=ident_d[:, :])

        # ---- per-head stacked score operand tiles ----
        # qhl[dc][e] = [q_hi(64); q_lo(64)] for head 2*dc+e
        # khh[dc][e] = [k_hi(64); k_hi(64)]   (duplicated)
        # klz[dc][e] = [k_lo(64); zeros(64)]
        qhl = [[cst.tile([128, SEQ], BF16, tag=f"qhl{dc}{e}", name=f"qhl{dc}{e}")
                for e in range(2)] for dc in range(2)]
        khh = [[cst.tile([128, SEQ], BF16, tag=f"khh{dc}{e}", name=f"khh{dc}{e}")
                for e in range(2)] for dc in range(2)]
        klz = [[cst.tile([128, SEQ], BF16, tag=f"klz{dc}{e}", name=f"klz{dc}{e}")
                for e in range(2)] for dc in range(2)]
        for dc in range(2):
            for e in range(2):
                nc.gpsimd.memset(klz[dc][e][64:128, :], 0.0)
        v_sb = cst.tile([128, NQT, DL], BF16, tag="v")
        ones_sb = cst.tile([128, 1], BF16, tag="ones")
        nc.gpsimd.memset(ones_sb[:, :], 1.0)

        def load_x_block(qc):
            """Stream one 512-col block of x (all MC chunks) on the SP queue."""
            xbh = xblk.tile([128, MC, 512], BF16, tag="xbh", name=f"xbh{qc}")
            xbl = xblk.tile([128, MC, 512], BF16, tag="xbl", name=f"xbl{qc}")
            nc.sync.dma_start(out=xbh[:, :, :], in_=xh_d[:, :, 512 * qc:512 * (qc + 1)])
            nc.sync.dma_start(out=xbl[:, :, :], in_=xl_d[:, :, 512 * qc:512 * (qc + 1)])
            return xbh, xbl

        def emit_proj_chunk(qc, xbh, xbl):
            cols = slice(512 * qc, 512 * (qc + 1))
            for wh_nm, wl_nm, dst_q in (("wqh", "wql", True), ("wkh", "wkl", False)):
                st_h = qkst.tile([128, 2, 512], BF16, tag=f"sh{wh_nm}", name="st_h")
                st_l = qkst.tile([128, 2, 512], BF16, tag=f"sl{wh_nm}", name="st_l")
                for dc in range(2):
                    ps = sp.tile([128, SCW], F32, tag="s", name="ps")
                    n = 0
                    for m in range(MC):
                        for lt, rt in ((wsb[wh_nm], xbh), (wsb[wh_nm], xbl),
                                       (wsb[wl_nm], xbh)):
                            nc.tensor.matmul(
                                ps[:, :512],
                                lt[:, m, 128 * dc:128 * (dc + 1)],
                                rt[:, m, :],
                                start=(n == 0), stop=(n == 3 * MC - 1))
                            n += 1
                    nc.scalar.copy(out=st_h[:, dc, :], in_=ps[:, :512])
                    nc.vector.tensor_sub(st_l[:, dc, :], ps[:, :512], st_h[:, dc, :])
                # rearrangement DMAs into stacked per-head tiles (gpsimd queue)
                for dc in range(2):
                    for e in range(2):
                        hs = slice(64 * e, 64 * (e + 1))
                        if dst_q:
                            nc.gpsimd.dma_start(out=qhl[dc][e][0:64, cols],
                                                in_=st_h[hs, dc, :])
                            nc.gpsimd.dma_start(out=qhl[dc][e][64:128, cols],
                                                in_=st_l[hs, dc, :])
                        else:
                            nc.gpsimd.dma_start(out=khh[dc][e][0:64, cols],
                                                in_=st_h[hs, dc, :])
                            nc.gpsimd.dma_start(out=khh[dc][e][64:128, cols],
                                                in_=st_h[hs, dc, :])
                            nc.gpsimd.dma_start(out=klz[dc][e][0:64, cols],
                                                in_=st_l[hs, dc, :])

        def emit_v(st, xbh):
            ps = misc.tile([128, 512], F32, tag="m", name="vps")
            for m in range(MC):
                nc.tensor.matmul(ps[:, :DL], xbh[:, m, 128 * (st % 4):128 * (st % 4) + 128],
                                 wsb["wvh"][:, m, :], start=(m == 0), stop=(m == MC - 1))
            nc.scalar.copy(out=v_sb[:, st, :], in_=ps[:, :DL])

        # ---- attention (software-pipelined: back-half of q-tile qi is
        # emitted after the front-half of qi+1 so the PE never stalls on a
        # fresh softmax) ----
        xblocks = [load_x_block(qc) for qc in range(4)]

        def emit_front(qi):
            if qi % 4 == 0:
                xbh, xbl = xblocks[qi // 4]
                emit_proj_chunk(qi // 4, xbh, xbl)
                emit_front.xbh = xbh
            emit_v(qi, emit_front.xbh)
            nkt = qi + 1          # causal k tiles
            kend = nkt * 128
            streams = []
            for hp in range(2):
                for e in range(2):
                    p_sb = pwork.tile([128, SEQ], BF16, tag="p", name="p_sb")
                    ncw = (kend + SCW - 1) // SCW
                    s_tiles = [None] * ncw
                    nms = []
                    qcols = slice(128 * qi, 128 * (qi + 1))
                    for ck in range(ncw):
                        cw = min(SCW, kend - SCW * ck)
                        sps = sp.tile([128, SCW], F32, tag="s", name="s_ps")
                        s_tiles[ck] = sps
                        has_mask = (SCW * ck <= 128 * qi < SCW * ck + cw)
                        for sub in range((cw + 511) // 512):
                            sw = min(512, cw - 512 * sub)
                            kcols = slice(SCW * ck + 512 * sub, SCW * ck + 512 * sub + sw)
                            nc.tensor.matmul(sps[:, 512 * sub:512 * sub + sw],
                                             qhl[hp][e][:, qcols], khh[hp][e][:, kcols],
                                             start=True, stop=False)
                            nc.tensor.matmul(sps[:, 512 * sub:512 * sub + sw],
                                             qhl[hp][e][:, qcols], klz[hp][e][:, kcols],
                                             start=False, stop=not has_mask)
                        if has_mask:
                            off = 128 * qi - SCW * ck
                            nc.tensor.matmul(sps[:, off:off + 128],
                                             ident_sb[:, :], mask_sb[:, :],
                                             start=False, stop=True)
                        nmc = small.tile([128, 1], F32, tag="nmc", name="nmc")
                        nc.vector.tensor_reduce(out=nmc, in_=sps[:, :cw],
                                                axis=mybir.AxisListType.X,
                                                op=mybir.AluOpType.max, negate=True)
                        nms.append(nmc)
                    nm = nms[0]
                    for ck in range(1, ncw):
                        nmg = small.tile([128, 1], F32, tag="nmg", name="nmg")
                        nc.vector.tensor_tensor(out=nmg, in0=nm, in1=nms[ck],
                                                op=mybir.AluOpType.min)
                        nm = nmg
                    for ck in range(ncw):
                        cw = min(SCW, kend - SCW * ck)
                        nc.scalar.activation(out=p_sb[:, SCW * ck:SCW * ck + cw],
                                             in_=s_tiles[ck][:, :cw],
                                             func=mybir.ActivationFunctionType.Exp,
                                             bias=nm, scale=1.0)
                    streams.append((hp, e, p_sb))
            return qi, nkt, streams

        def emit_back(ctx):
            qi, nkt, streams = ctx
            attn_cat = work.tile([128, DL], BF16, tag="acat")
            av_pairs = {}
            for hp, e, p_sb in streams:
                    h_local = 2 * hp + e
                    # P^T via PE transpose, PTG k-tiles per group
                    pt_sb = ptwork.tile([128, SEQ], BF16, tag="pt", name="pt_sb")
                    for g in range((nkt + PTG - 1) // PTG):
                        n_in_g = min(PTG, nkt - PTG * g)
                        ptps = ptp.tile([128, 128 * PTG], BF16, tag="ptps", name="ptps")
                        for j in range(n_in_g):
                            kt_i = PTG * g + j
                            nc.tensor.transpose(ptps[:, 128 * j:128 * (j + 1)],
                                                p_sb[:, 128 * kt_i:128 * (kt_i + 1)],
                                                ident_sb)
                        if g % 2 == 0:
                            nc.vector.tensor_copy(out=pt_sb[:, 128 * PTG * g:128 * PTG * g + 128 * n_in_g],
                                                  in_=ptps[:, :128 * n_in_g])
                        else:
                            nc.scalar.copy(out=pt_sb[:, 128 * PTG * g:128 * PTG * g + 128 * n_in_g],
                                           in_=ptps[:, :128 * n_in_g])
                    # attn @ V with fp32 accumulation
                    if e == 0:
                        av_pairs[hp] = misc.tile([128, 512], F32, tag="m", name="av_pair")
                    av_pair = av_pairs[hp]
                    for kt_i in range(nkt):
                        nc.tensor.matmul(av_pair[:, 65 * e:65 * e + 64],
                                         pt_sb[:, 128 * kt_i:128 * (kt_i + 1)],
                                         v_sb[:, kt_i, 64 * h_local:64 * (h_local + 1)],
                                         start=(kt_i == 0), stop=(kt_i == nkt - 1))
                    for kt_i in range(nkt):
                        nc.tensor.matmul(av_pair[:, 65 * e + 64:65 * e + 65],
                                         pt_sb[:, 128 * kt_i:128 * (kt_i + 1)],
                                         ones_sb[:, :],
                                         start=(kt_i == 0), stop=(kt_i == nkt - 1))
                    inv = small.tile([128, 1], F32, tag="inv", name="inv")
                    nc.vector.reciprocal(out=inv, in_=av_pair[:, 65 * e + 64:65 * e + 65])
                    nc.scalar.activation(
                        out=attn_cat[:, 64 * h_local:64 * (h_local + 1)],
                        in_=av_pair[:, 65 * e:65 * e + 64],
                        func=mybir.ActivationFunctionType.Copy, scale=inv)
            # ---- output projection for this q tile ----
            acT_ps = ptp.tile([128, 128 * PTG], BF16, tag="ptps", name="acT_ps")
            nc.tensor.transpose(acT_ps[:, 0:128], attn_cat[:, 0:128], ident_sb)
            nc.tensor.transpose(acT_ps[:, 128:256], attn_cat[:, 128:256], ident_sb)
            acT = work.tile([128, 256], BF16, tag="acT")
            nc.vector.tensor_copy(out=acT[:, :], in_=acT_ps[:, :256])
            out_sb = work.tile([128, DM], F32, tag="osb")
            for nc_i in range(2):
                ops = misc.tile([128, 512], F32, tag="m", name="ops")
                for mlc in range(2):
                    nc.tensor.matmul(ops[:, :512], acT[:, 128 * mlc:128 * (mlc + 1)],
                                     poT_sb[:, mlc, 512 * nc_i:512 * (nc_i + 1)],
                                     start=(mlc == 0), stop=(mlc == 1))
                if nc_i == 0:
                    nc.scalar.copy(out=out_sb[:, 0:512], in_=ops[:, :512])
                else:
                    nc.vector.tensor_copy(out=out_sb[:, 512:1024], in_=ops[:, :512])
            nc.gpsimd.dma_start(out=out_d[128 * qi:128 * (qi + 1), :], in_=out_sb)

        pend = []
        for qi in range(NQT):
            pend.append(emit_front(qi))
            if len(pend) > 2:
                emit_back(pend.pop(0))
        for ctx_f in pend:
            emit_back(ctx_f)

    nc.compile()
    return nc


def _bf16(a):
    return a.astype(ml_dtypes.bfloat16)


def _split(a):
    hi = _bf16(a)
    lo = _bf16(a - hi.astype(np.float32))
    return hi, lo


def _prep_inputs(x, p_q, p_k, p_v, p_o):
    """Build the 8 per-core input maps."""
    per_batch = []
    for b in range(2):
        xT = np.ascontiguousarray(x[b].T).astype(np.float32)  # [1024, 2048]
        xh, xl = _split(xT)
        per_batch.append((np.ascontiguousarray(xh.reshape(MC, 128, SEQ).transpose(1, 0, 2)),
                          np.ascontiguousarray(xl.reshape(MC, 128, SEQ).transpose(1, 0, 2))))

    mask = np.zeros((128, 128), np.float32)
    iu = np.triu_indices(128, 1)
    mask[iu] = NEG
    mask = _bf16(mask)
    ident = np.eye(128, dtype=ml_dtypes.bfloat16)

    per_group = []
    for g in range(4):
        rows = slice(DL * g, DL * (g + 1))
        wqT = np.ascontiguousarray((p_q[rows] / math.sqrt(DH)).T).astype(np.float32)
        wkT = np.ascontiguousarray(p_k[rows].T).astype(np.float32)
        wvT = np.ascontiguousarray(p_v[rows].T).astype(np.float32)
        poT = np.ascontiguousarray(p_o[:, rows].T).astype(np.float32)
        wqh, wql = _split(wqT)
        wkh, wkl = _split(wkT)
        def _pm(a):
            return np.ascontiguousarray(a.reshape(MC, 128, DL).transpose(1, 0, 2))
        per_group.append(dict(
            wqh=_pm(wqh), wql=_pm(wql), wkh=_pm(wkh), wkl=_pm(wkl),
            wvh=_pm(_bf16(wvT)),
            poT=np.ascontiguousarray(_bf16(poT).reshape(2, 128, DM).transpose(1, 0, 2)),
        ))

    in_maps = []
    for c in range(8):
        b, g = c // 4, c % 4
        m = dict(per_group[g])
        m["xh"], m["xl"] = per_batch[b]
        m["mask"] = mask
        m["ident"] = ident
        in_maps.append(m)
    return in_maps


def kernel(x, p_q, p_k, p_v, p_o):
    if "nc" not in _CACHE:
        _CACHE["nc"] = build_nc()
    nc = _CACHE["nc"]
    in_maps = _prep_inputs(np.asarray(x), np.asarray(p_q), np.asarray(p_k),
                           np.asarray(p_v), np.asarray(p_o))
    res = run_bass_kernel_spmd(nc, in_maps, core_ids=list(range(8)))
    parts = [r["out_part"].astype(np.float32) for r in res.results]
    out = np.stack([parts[0] + parts[1] + parts[2] + parts[3],
                    parts[4] + parts[5] + parts[6] + parts[7]])
    return out.astype(np.float32)
